# Initial kernel scaffold
#
"""Trainium2 Bass kernel for causal multi-head attention with RoPE.

Model: B=2, S=2048, H=2048, 16 heads x 128 head-dim.
  qkv = x @ w_qkv.T ; RoPE(q, k); causal softmax(q k^T / sqrt(dh)) @ v; out = attn @ w_o.T

Sharding: tensor-parallel over heads. Each of the 8 cores owns 2 heads:
it computes q/k/v projections for its heads (w_qkv row slices), runs
flash-style causal attention for them, and applies its slice of w_o
columns, producing a partial [B,S,H] output. The host sums the 8
partials in fp32 (the all-reduce "unshard" of the TP strategy).

On-core layout choices:
  - Q,K projected in natural [token, dim] layout so RoPE's rotate-half
    pairs sit at free-dim offsets (cross-partition DVE reads are
    rejected by the walrus verifier), then PE-transposed to [dim, token]
    for the scores matmul.
  - Scores computed transposed (S^T[kt, qt]) so the exp'd probabilities
    feed the PV matmul directly with no per-block transpose; the softmax
    denominator comes from an M=1 ones-matmul accumulated in PSUM, and
    is folded in after PV via reciprocal + gpsimd partition_broadcast.
  - No max-subtraction in softmax: inputs are unit-scale gaussians, so
    scaled scores are O(10) and exp stays comfortably inside fp32/bf16.
  - All matmuls in bf16 with fp32 PSUM accumulation.
"""

import contextlib
import math

import numpy as np
import ml_dtypes

B = 2
S = 2048
HID = 2048
NH = 16
DH = 128
NCORES = 8
HPC = NH // NCORES  # heads per core
CH = 512            # chunk (free-dim) size
NEG = -1.0e30

_STATE = {}

# tuning knobs (read at build time)
CFG = {"skew": 2, "pt_bufs": 4, "psa": 3, "psr_own_bank": True,
       "any_ot": True}


# ----------------------------------------------------------------------------
# device kernel
# ----------------------------------------------------------------------------

def _emit_body(nc, r, seq_len, parts="all"):
    """Emit one full pass of the computation. `r` holds pools + consts."""
    import concourse.bass_isa as bass_isa
    import concourse.mybir as mybir

    bf16 = mybir.dt.bfloat16
    f32 = mybir.dt.float32
    Exp = mybir.ActivationFunctionType.Exp
    NT = seq_len // 128
    TC = seq_len // CH
    NHB = HID // 128
    SCALE = 1.0 / math.sqrt(DH)

    def oproj_fillers(b, qi, at_pair):
        """Emitters for chunk qi's output projection, one (tt, oc) tile
        each — interleaved into the next chunk's attention loop as PE
        filler so exp latency on ACT never stalls the PE queue."""
        tiles = {}

        def make(tt, oc):
            def emit():
                if tt not in tiles:
                    tiles[tt] = r.opool.tile([128, HID], bf16, tag="ot",
                                             name="ot")
                ot = tiles[tt]
                pop = r.psC.tile([128, CH], f32, tag="C", name="pop")
                for h in range(2):
                    nc.tensor.matmul(
                        pop[:],
                        at_pair[h][:, tt * 128:(tt + 1) * 128],
                        r.wo_sb[:, h, oc * CH:(oc + 1) * CH],
                        start=(h == 0), stop=(h == 1),
                    )
                if CFG.get("any_ot"):
                    nc.any.tensor_copy(ot[:, oc * CH:(oc + 1) * CH], pop[:])
                else:
                    nc.vector.tensor_copy(ot[:, oc * CH:(oc + 1) * CH], pop[:])
                if oc == HID // CH - 1:
                    nc.sync.dma_start(
                        r.out_d[b, qi * CH + tt * 128:
                                qi * CH + (tt + 1) * 128, :],
                        ot[:],
                    )
            return emit

        return [make(tt, oc) for tt in range(4) for oc in range(HID // CH)]

    for b in range(B if parts == "all" else 1):
        # ---------------- QKV projection + RoPE ----------------
        qks = []
        for nm in ("q0t", "q1t", "k0t", "k1t"):
            qks.append(r.qkpool.tile([128, seq_len], bf16, tag="qkt", name=nm))
        q0t, q1t, k0t, k1t = qks
        vt = r.vpool.tile([128, NT, 2 * DH], bf16, tag="vt", name="vt")
        xTb = r.xT[b].rearrange("(n p) t -> p n t", p=128)

        def emit_transposes(qr, j):
            for o, dst in enumerate((q0t, q1t, k0t, k1t)):
                ptr = r.psC.tile([128, 128], bf16, tag="C", name="ptr")
                nc.tensor.transpose(
                    ptr[:], qr[:, o * 128:(o + 1) * 128], r.ident[:])
                nc.vector.tensor_copy(
                    dst[:, j * 128:(j + 1) * 128], ptr[:])

        pending = None  # transposes run one tile behind their RoPE chain
        for tc4 in range(TC):
            xts = r.xpool.tile([128, NHB, CH], bf16, tag="xt", name="xts")
            nc.sync.dma_start(
                xts[:], xTb[:, :, tc4 * CH:(tc4 + 1) * CH])
            for tt in range(4):
                j = 4 * tc4 + tt
                psqk = r.psA.tile([128, 4 * DH], f32, tag="A", name="psqk")
                psv = r.psB.tile([128, 2 * DH], f32, tag="B", name="psv")
                for hb in range(NHB):
                    lhs = xts[:, hb, tt * 128:(tt + 1) * 128]
                    nc.tensor.matmul(
                        psqk[:], lhs, r.wqk_sb[:, hb, :],
                        start=(hb == 0), stop=(hb == NHB - 1),
                    )
                    nc.tensor.matmul(
                        psv[:], lhs, r.wv_sb[:, hb, :],
                        start=(hb == 0), stop=(hb == NHB - 1),
                    )
                if pending is not None:
                    emit_transposes(*pending)
                nc.scalar.copy(vt[:, j, :], psv[:])
                # RoPE on the whole [128, 512] q0|q1|k0|k1 block at once:
                # strided views pair (d, d+64) within each 128-block.
                ps4 = psqk.rearrange("p (o h d) -> p o h d", h=2, d=64)
                sin4 = r.sinc[:, j, :].rearrange("p (o h d) -> p o h d", h=2, d=64)
                t1 = r.tpool.tile([128, CH], f32, tag="t1", name="t1")
                t14 = t1.rearrange("p (o h d) -> p o h d", h=2, d=64)
                t2 = r.tpool.tile([128, CH], f32, tag="t2", name="t2")
                nc.vector.tensor_mul(t14[:, :, 0, :], ps4[:, :, 1, :],
                                     sin4[:, :, 0, :])
                nc.vector.tensor_mul(t14[:, :, 1, :], ps4[:, :, 0, :],
                                     sin4[:, :, 1, :])
                nc.vector.tensor_mul(t2[:], psqk[:], r.cosc[:, j, :])
                qr = r.qrpool.tile([128, CH], bf16, tag="qr", name="qr")
                nc.vector.tensor_add(qr[:], t1[:], t2[:])
                pending = (qr, j)
        emit_transposes(*pending)

        if parts == "qkv":
            continue
        # ---------------- attention + output projection --------
        prev_at = None
        fillers = []
        for qi in range(TC):
            if prev_at is not None:
                fillers = oproj_fillers(b, qi - 1, prev_at)
            at_pair = []
            for h, (Q, K) in enumerate(((q0t, k0t), (q1t, k1t))):
                pso = r.psB.tile([128, CH], f32, tag="B", name="pso")
                psr = r.psD.tile([128, CH], f32, tag="D", name="psr")
                nj = 4 * qi + 4

                def emit_scores(jb):
                    # Diagonal blocks only produce nonzero probabilities for
                    # qt >= kt; narrow all work to that column subrange.
                    r8 = jb - 4 * qi
                    lo = 128 * r8 if r8 > 0 else 0
                    sub = slice(lo, CH)
                    pss = r.psA.tile([128, CH], f32, tag="A", name="pss")
                    nc.tensor.matmul(
                        pss[:, sub], K[:, jb * 128:(jb + 1) * 128],
                        Q[:, qi * CH + lo:(qi + 1) * CH],
                        start=True, stop=True,
                    )
                    if r8 >= 0:
                        nc.vector.tensor_add(
                            pss[:, sub], pss[:, sub], r.mask_sb[:, r8, sub])
                    pt = r.ptpool.tile([128, CH], bf16, tag="pt", name="pt")
                    nc.scalar.activation(pt[:, sub], pss[:, sub], Exp,
                                         scale=SCALE)
                    return pt, lo

                def emit_pv(jb, pt, lo):
                    sub = slice(lo, CH)
                    nc.tensor.matmul(
                        pso[:, sub], vt[:, jb, h * DH:(h + 1) * DH], pt[:, sub],
                        start=(jb == 0), stop=(jb == nj - 1))
                    # rowsum via M=128 all-ones stationary: every psum
                    # partition receives the same column sums, so no
                    # post-hoc partition broadcast is needed.
                    nc.tensor.matmul(
                        psr[:, sub], r.ones[:], pt[:, sub],
                        start=(jb == 0), stop=(jb == nj - 1))

                # scores run two blocks ahead of PV so the PE never waits
                # on the mask+exp chain of the block it just scored; oproj
                # matmuls from the previous chunk fill remaining PE slack.
                SKEW = CFG["skew"]
                queue = []
                for jb in range(nj):
                    queue.append((jb, emit_scores(jb)))
                    if fillers:
                        fillers.pop(0)()
                    if len(queue) > SKEW:
                        pj, args = queue.pop(0)
                        emit_pv(pj, *args)
                for pj, args in queue:
                    emit_pv(pj, *args)

                rsb = r.rspool.tile([128, CH], f32, tag="rsb", name="rsb")
                nc.vector.reciprocal(rsb[:], psr[:])
                at = r.atpool.tile([128, CH], bf16, tag="at", name="at")
                nc.vector.tensor_mul(at[:], pso[:], rsb[:])
                at_pair.append(at)
            for f in fillers:
                f()
            fillers = []
            prev_at = at_pair
        for f in oproj_fillers(b, TC - 1, prev_at):
            f()


class _Res:
    pass


def build_nc(seq_len=S, loop_n=1, parts="all"):
    """Build the per-core program. loop_n>1 wraps the body in a hardware
    loop — a timing-only variant used to measure per-iteration device
    time through the noisy dispatch path."""
    import concourse.mybir as mybir
    import concourse.tile as tile
    from concourse import bacc
    from concourse.masks import make_identity

    bf16 = mybir.dt.bfloat16
    f32 = mybir.dt.float32
    NT = seq_len // 128

    nc = bacc.Bacc("TRN2", target_bir_lowering=False, debug=False)

    r = _Res()
    r.xT = nc.dram_tensor("xt", [B, HID, seq_len], bf16, kind="ExternalInput")
    wqk = nc.dram_tensor("wqk", [HID, 4 * DH], bf16, kind="ExternalInput")
    wv = nc.dram_tensor("wv", [HID, 2 * DH], bf16, kind="ExternalInput")
    wo = nc.dram_tensor("wo", [2 * DH, HID], bf16, kind="ExternalInput")
    rope_d = {}
    for nm in ("cosc", "sinc"):
        rope_d[nm] = nc.dram_tensor(nm, [seq_len, 4 * DH], bf16,
                                    kind="ExternalInput")
    mask_d = nc.dram_tensor("masks", [4, 128, CH], f32, kind="ExternalInput")
    r.out_d = nc.dram_tensor("out", [B, seq_len, HID], bf16,
                             kind="ExternalOutput")

    with tile.TileContext(nc) as tc:
        with (
            tc.tile_pool(name="consts", bufs=1) as cpool,
            tc.tile_pool(name="x", bufs=2) as xpool,
            tc.tile_pool(name="qk", bufs=8) as qkpool,
            tc.tile_pool(name="v", bufs=2) as vpool,
            tc.tile_pool(name="pt", bufs=CFG["pt_bufs"]) as ptpool,
            tc.tile_pool(name="at", bufs=4) as atpool,
            tc.tile_pool(name="tmp", bufs=2) as tpool,
            tc.tile_pool(name="qr", bufs=3) as qrpool,
            tc.tile_pool(name="rs", bufs=2) as rspool,
            tc.tile_pool(name="o", bufs=2) as opool,
            tc.tile_pool(name="psA", bufs=CFG["psa"], space="PSUM") as psA,
            tc.tile_pool(name="psB", bufs=2, space="PSUM") as psB,
            tc.tile_pool(name="psC", bufs=2, space="PSUM") as psC,
            tc.tile_pool(name="psD", bufs=1, space="PSUM") as psD,
        ):
            r.xpool, r.qkpool, r.vpool, r.ptpool = xpool, qkpool, vpool, ptpool
            r.atpool, r.tpool, r.qrpool, r.rspool = atpool, tpool, qrpool, rspool
            r.opool = opool
            r.psA, r.psB, r.psC, r.psD = psA, psB, psC, psD

            r.wqk_sb = cpool.tile([128, HID // 128, 4 * DH], bf16, name="wqk_sb")
            nc.sync.dma_start(r.wqk_sb[:], wqk.rearrange("(n p) o -> p n o", p=128))
            r.wv_sb = cpool.tile([128, HID // 128, 2 * DH], bf16, name="wv_sb")
            nc.sync.dma_start(r.wv_sb[:], wv.rearrange("(n p) o -> p n o", p=128))
            r.wo_sb = cpool.tile([128, 2, HID], bf16, name="wo_sb")
            nc.sync.dma_start(r.wo_sb[:], wo.rearrange("(n p) o -> p n o", p=128))
            for nm in ("cosc", "sinc"):
                t = cpool.tile([128, NT, 4 * DH], bf16, name=nm)
                nc.sync.dma_start(t[:], rope_d[nm].rearrange("(n p) d -> p n d", p=128))
                setattr(r, nm, t)
            r.mask_sb = cpool.tile([128, 4, CH], f32, name="mask_sb")
            nc.sync.dma_start(r.mask_sb[:], mask_d.rearrange("n p o -> p n o"))
            r.ident = cpool.tile([128, 128], bf16, name="ident")
            make_identity(nc, r.ident[:])
            r.ones = cpool.tile([128, 128], bf16, name="ones")
            nc.gpsimd.memset(r.ones[:], 1.0)

            loop_ctx = (tc.For_i(0, loop_n, 1) if loop_n > 1
                        else contextlib.nullcontext())
            with loop_ctx:
                _emit_body(nc, r, seq_len, parts)

    nc.compile()
    return nc


# ----------------------------------------------------------------------------
# host-side sharding / tables
# ----------------------------------------------------------------------------

def host_tables(seq_len=S):
    bf = ml_dtypes.bfloat16
    inv = 1.0 / (10000.0 ** (np.arange(0, DH, 2, dtype=np.float64) / DH))
    ang = np.arange(seq_len, dtype=np.float64)[:, None] * inv[None, :]  # [S, 64]
    cos = np.cos(ang)
    sin = np.sin(ang)
    cos_td = np.concatenate([cos, cos], axis=1)                  # [S, 128]
    ssin_td = np.concatenate([-sin, sin], axis=1)                # signed swap mult
    tabs = {
        "cosc": np.ascontiguousarray(np.tile(cos_td, (1, 4))).astype(bf),
        "sinc": np.ascontiguousarray(np.tile(ssin_td, (1, 4))).astype(bf),
    }
    p = np.arange(128)[:, None]
    f = np.arange(CH)[None, :]
    masks = np.stack(
        [np.where(p + 128 * ri <= f, 0.0, NEG) for ri in range(4)]
    ).astype(np.float32)
    tabs["masks"] = masks
    return tabs


def host_in_maps(x, w_qkv, w_o, seq_len=S):
    bf = ml_dtypes.bfloat16
    x = np.asarray(x, dtype=np.float32)
    w_qkv = np.asarray(w_qkv, dtype=np.float32)
    w_o = np.asarray(w_o, dtype=np.float32)
    xT = np.ascontiguousarray(x.transpose(0, 2, 1)).astype(bf)
    tabs = host_tables(seq_len)
    maps = []
    for c in range(NCORES):
        h0 = HPC * c
        rows = []
        for base in (0, HID):  # q rows, then k rows
            for h in range(h0, h0 + HPC):
                rows.append(w_qkv[base + h * DH:base + (h + 1) * DH])
        wqk_c = np.ascontiguousarray(np.concatenate(rows, axis=0).T).astype(bf)
        vrows = [w_qkv[2 * HID + h * DH:2 * HID + (h + 1) * DH]
                 for h in range(h0, h0 + HPC)]
        wv_c = np.ascontiguousarray(np.concatenate(vrows, axis=0).T).astype(bf)
        wo_c = np.ascontiguousarray(
            w_o[:, h0 * DH:(h0 + HPC) * DH].T).astype(bf)
        maps.append({
            "xt": xT, "wqk": wqk_c, "wv": wv_c, "wo": wo_c,
            "cosc": tabs["cosc"], "sinc": tabs["sinc"],
            "masks": tabs["masks"],
        })
    return maps


def kernel(x, w_qkv, w_o):
    from concourse import bass_utils

    if "nc" not in _STATE:
        _STATE["nc"] = build_nc(S)
    nc = _STATE["nc"]
    in_maps = host_in_maps(x, w_qkv, w_o, S)
    res = bass_utils.run_bass_kernel_spmd(
        nc, in_maps, core_ids=list(range(NCORES)))
    out = np.zeros((B, S, HID), dtype=np.float32)
    for r in res.results:
        out += np.asarray(r["out"], dtype=np.float32)
    return out



# revision 3
# speedup vs baseline: 1.0058x; 1.0058x over previous
"""Trainium2 Bass kernel for causal multi-head attention with RoPE (v2).

Model: B=2, S=2048, H=2048, 16 heads x 128 head-dim.
  qkv = x @ w_qkv.T ; RoPE(q, k); causal softmax(q k^T / sqrt(dh)) @ v; out = attn @ w_o.T

Sharding: tensor-parallel over heads. Each of the 8 cores owns 2 heads.
The host sums the 8 partial [B,S,H] outputs in fp32.

v2 structural changes vs v1:
  - Q^T and K^T are produced directly by the projection matmuls
    (stationary = w column block, moving = x^T chunk), so the PE
    transposes and their PSUM->SBUF copies are gone. RoPE is applied in
    [dim, token] layout; the rotate-half partner lives 64 partitions
    away, which the DVE can read directly because the input is in PSUM
    (the base-partition equality rule only binds SBUF operands).
  - The causal mask is accumulated into the scores PSUM tile by an
    identity x mask matmul instead of a DVE add, shortening the
    scores -> exp chain to PE -> ACT.
  - RoPE tables are [dh, S] (no 4x duplication).
"""

import contextlib
import math

import numpy as np
import ml_dtypes

B = 2
S = 2048
HID = 2048
NH = 16
DH = 128
NCORES = 8
HPC = NH // NCORES  # heads per core
CH = 512            # chunk (free-dim) size
NEG = -1.0e30

_STATE = {}

CFG = {"skew": 3, "pt_bufs": 8, "psa": 3, "psc": 2, "pe_mask": False,
       "rs_fold": True, "rs_fold_diag": True, "ot_eng": "vector",
       "vt_eng": "scalar", "desc_qi": False, "qkv_eager": 1}


# ----------------------------------------------------------------------------
# device kernel
# ----------------------------------------------------------------------------

def _emit_body(nc, r, seq_len, parts="all"):
    """Emit one full pass of the computation. `r` holds pools + consts."""
    import concourse.bass_isa as bass_isa
    import concourse.mybir as mybir

    bf16 = mybir.dt.bfloat16
    f32 = mybir.dt.float32
    Exp = mybir.ActivationFunctionType.Exp
    NT = seq_len // 128
    TC = seq_len // CH
    NHB = HID // 128
    SCALE = 1.0 / math.sqrt(DH)

    r.xts_pend = {}

    def oproj_fillers(b, qi, at_pair):
        """Emitters for chunk qi's output projection, one (tt, oc) tile
        each — interleaved into the next chunk's attention loop as PE
        filler so exp latency on ACT never stalls the PE queue."""
        tiles = {}

        def make(tt, oc):
            def emit():
                if tt not in tiles:
                    tiles[tt] = r.opool.tile([128, HID], bf16, tag="ot",
                                             name="ot")
                ot = tiles[tt]
                pop = r.psC.tile([128, CH], f32, tag="C", name="pop")
                for h in range(2):
                    nc.tensor.matmul(
                        pop[:],
                        at_pair[h][:, tt * 128:(tt + 1) * 128],
                        r.wo_sb[:, h, oc * CH:(oc + 1) * CH],
                        start=(h == 0), stop=(h == 1),
                    )
                oe = CFG["ot_eng"]
                if oe == "alt":
                    oe = ("vector", "scalar")[oc % 2]
                if oe == "scalar":
                    nc.scalar.copy(ot[:, oc * CH:(oc + 1) * CH], pop[:])
                else:
                    getattr(nc, oe).tensor_copy(
                        ot[:, oc * CH:(oc + 1) * CH], pop[:])
                if oc == HID // CH - 1:
                    nc.sync.dma_start(
                        r.out_d[b, qi * CH + tt * 128:
                                qi * CH + (tt + 1) * 128, :],
                        ot[:],
                    )
            return emit

        return [make(tt, oc) for tt in range(4) for oc in range(HID // CH)]

    for b in range(B if parts == "all" else 1):
        # ---------------- QKV projection + RoPE ----------------
        # Per-chunk tiles so the attention phase's dependency on chunk
        # j's K/V resolves as soon as that chunk is written (tile-
        # granularity tracking would otherwise stall the first scores
        # matmul on the last chunk's RoPE).
        qkc = []   # qkc[c] = (q0, q1, k0, k1) tiles [128, CH]
        vtc = []   # vtc[c] = [128, 4, 2*DH]

        def qkv_chunk_units(b, c, blocks, vt):
            """Closures emitting chunk c's QKV work: 4 psq groups (+RoPE)
            and 4 psv groups. Run immediately for early chunks; fed to
            attention(qi=0) as PE fillers for the later ones."""
            cs = slice(c * CH, (c + 1) * CH)
            state = {}

            def get_xts():
                if "xts" not in state:
                    xts = r.xts_pend.pop((b, c), None)
                    if xts is None:
                        xts = r.xpool.tile([128, NHB, CH], bf16, tag="xt",
                                           name="xts")
                        nc.sync.dma_start(xts[:], r.xT[b, c])
                    # prefetch 2 chunks ahead (crossing into the next
                    # batch) so the load is never behind attention-phase
                    # output DMAs in the queue
                    pb, pc = b, c + 2
                    if pc >= TC:
                        pb, pc = b + 1, pc - TC
                    if pb < B and (pb, pc) not in r.xts_pend:
                        nxt = r.xpool.tile([128, NHB, CH], bf16, tag="xt",
                                           name="xts")
                        nc.sync.dma_start(nxt[:], r.xT[pb % B, pc])
                        r.xts_pend[(pb, pc)] = nxt
                    state["xts"] = xts
                return state["xts"]

            def qk_unit(db):
                def emit():
                    xts = get_xts()
                    dst = r.qkpool.tile([128, CH], bf16, tag="qkt",
                                        name=f"qk{db}")
                    blocks[db] = dst
                    psq = r.psA.tile([128, CH], f32, tag="A", name="psq")
                    for hb in range(NHB):
                        nc.tensor.matmul(
                            psq[:],
                            r.wqk_sb[:, hb, db * 128:(db + 1) * 128],
                            xts[:, hb, :],
                            start=(hb == 0), stop=(hb == NHB - 1),
                        )
                    t1 = r.tpool.tile([128, CH], f32, tag="t1", name="t1")
                    t2 = r.tpool.tile([128, CH], f32, tag="t2", name="t2")
                    nc.vector.tensor_mul(t1[0:64, :], psq[64:128, :],
                                         r.ssinT[0:64, cs])
                    nc.vector.tensor_mul(t1[64:128, :], psq[0:64, :],
                                         r.ssinT[64:128, cs])
                    nc.vector.tensor_mul(t2[:], psq[:], r.cosT[:, cs])
                    nc.vector.tensor_add(dst[:], t1[:], t2[:])
                return emit

            def v_unit(tt):
                def emit():
                    xts = get_xts()
                    psv = r.psB.tile([128, 2 * DH], f32, tag="B", name="psv")
                    for hb in range(NHB):
                        nc.tensor.matmul(
                            psv[:],
                            xts[:, hb, tt * 128:(tt + 1) * 128],
                            r.wv_sb[:, hb, :],
                            start=(hb == 0), stop=(hb == NHB - 1),
                        )
                    if CFG["vt_eng"] == "scalar":
                        nc.scalar.copy(vt[:, tt, :], psv[:])
                    else:
                        getattr(nc, CFG["vt_eng"]).tensor_copy(
                            vt[:, tt, :], psv[:])
                return emit

            return [qk_unit(db) for db in range(4)] + \
                   [v_unit(tt) for tt in range(4)]

        qkv_fillers = []
        n_eager = TC if (CFG["desc_qi"] or parts == "qkv") else CFG["qkv_eager"]
        for c in range(TC):
            blocks = [None] * 4
            vt = r.vpool.tile([128, 4, 2 * DH], bf16, tag="vt", name="vt")
            qkc.append(blocks)
            vtc.append(vt)
            units = qkv_chunk_units(b, c, blocks, vt)
            if c < n_eager:
                for u in units:
                    u()
            else:
                qkv_fillers.extend(units)

        if parts == "qkv":
            for u in qkv_fillers:
                u()
            continue
        # ---------------- attention + output projection --------
        # Descending qi: the first chunk processed (qi=TC-1) is the one
        # with enough full-size blocks to self-hide exp latency; smaller
        # chunks get oproj fillers from the previously processed chunk.
        prev_at = None
        prev_qi = None
        fillers = []
        for qi in (range(TC - 1, -1, -1) if CFG["desc_qi"] else range(TC)):
            if prev_at is not None:
                fillers = oproj_fillers(b, prev_qi, prev_at)
            elif qkv_fillers:
                # first chunk processed: late QKV chunks serve as fillers
                fillers = qkv_fillers
                qkv_fillers = []
            at_pair = []
            for h in range(2):
                pso = r.psB.tile([128, CH], f32, tag="B", name="pso")
                psr = r.psD.tile([128, CH], f32, tag="D", name="psr")
                nj = 4 * qi + 4
                Q = qkc[qi][h]
                # rowsum folding state: full (non-diagonal) pt blocks are
                # pre-summed 4-at-a-time on the DVE, so the PE does one
                # ones-matmul per group of 4 instead of per block.
                fold = CFG["rs_fold"]
                fold_diag = CFG["rs_fold_diag"]
                rs_group = []       # pt tiles awaiting fold
                rs_ready = []       # s12 tiles awaiting their psr matmul
                rs_first = [True]
                rs_diag = []        # diagonal pt tiles (narrowed)

                def rs_matmul(mv, sub, stop, start=None):
                    nc.tensor.matmul(
                        psr[:, sub], r.ones[:], mv,
                        start=rs_first[0] if start is None else start,
                        stop=stop)
                    rs_first[0] = False

                def rs_flush_ready(stop=False):
                    while rs_ready:
                        s12 = rs_ready.pop(0)
                        last = stop and not rs_ready and not rs_group
                        rs_matmul(s12[:], slice(0, CH), last)

                def rs_fold_group():
                    pa, pb, pc, pd = rs_group
                    del rs_group[:]
                    s1 = r.spool.tile([128, CH], bf16, tag="s", name="s1")
                    s2 = r.spool.tile([128, CH], bf16, tag="s", name="s2")
                    s12 = r.spool.tile([128, CH], bf16, tag="s", name="s12")
                    nc.vector.tensor_add(s1[:], pa[:], pb[:])
                    nc.vector.tensor_add(s2[:], pc[:], pd[:])
                    nc.vector.tensor_add(s12[:], s1[:], s2[:])
                    rs_ready.append(s12)

                def emit_scores(jb):
                    # Diagonal blocks only produce nonzero probabilities for
                    # qt >= kt; narrow all work to that column subrange.
                    r8 = jb - 4 * qi
                    lo = 128 * r8 if r8 > 0 else 0
                    sub = slice(lo, CH)
                    K = qkc[jb // 4][2 + h]
                    pss = r.psA.tile([128, CH], f32, tag="A", name="pss")
                    pe_mask = CFG["pe_mask"]
                    nc.tensor.matmul(
                        pss[:, sub], K[:, (jb % 4) * 128:(jb % 4 + 1) * 128],
                        Q[:, lo:CH],
                        start=True, stop=(r8 < 0 or not pe_mask),
                    )
                    if r8 >= 0:
                        if pe_mask:
                            # mask = I^T @ mask_block accumulated in PSUM
                            nc.tensor.matmul(
                                pss[:, sub], r.ident[:], r.mask_sb[:, r8, sub],
                                start=False, stop=True,
                            )
                        else:
                            nc.vector.tensor_add(
                                pss[:, sub], pss[:, sub], r.mask_sb[:, r8, sub])
                    pt = r.ptpool.tile([128, CH], bf16, tag="pt", name="pt")
                    nc.scalar.activation(pt[:, sub], pss[:, sub], Exp,
                                         scale=SCALE)
                    return pt, lo

                def emit_pv(jb, pt, lo):
                    sub = slice(lo, CH)
                    nc.tensor.matmul(
                        pso[:, sub],
                        vtc[jb // 4][:, jb % 4, h * DH:(h + 1) * DH],
                        pt[:, sub],
                        start=(jb == 0), stop=(jb == nj - 1))
                    # rowsum via M=128 all-ones stationary: every psum
                    # partition receives the same column sums.
                    r8 = jb - 4 * qi
                    if not fold:
                        nc.tensor.matmul(
                            psr[:, sub], r.ones[:], pt[:, sub],
                            start=(jb == 0), stop=(jb == nj - 1))
                        return
                    if r8 < 0:
                        rs_group.append(pt)
                        if len(rs_group) == 4:
                            # emit the previous group's matmul first so the
                            # PE never waits on adds that just got queued
                            rs_flush_ready()
                            rs_fold_group()
                    elif not fold_diag:
                        # diagonal blocks: narrowed, summed directly
                        rs_flush_ready()
                        rs_matmul(pt[:, sub], sub, stop=(jb == nj - 1))
                    else:
                        # diagonal blocks: DVE-chain the [128:] suffix, one
                        # small matmul for block 0's [0:128) prefix; the
                        # suffix matmul is deferred to rs_finalize so tail
                        # fillers cover the add-chain latency
                        rs_diag.append(pt)
                        if r8 == 0:
                            rs_flush_ready()
                            rs_matmul(pt[:, 0:128], slice(0, 128), False)
                        elif r8 == 1:
                            ds = r.spool.tile([128, CH], bf16, tag="s",
                                              name="ds")
                            rs_diag.append(ds)
                            nc.vector.tensor_add(
                                ds[:, 128:CH], rs_diag[0][:, 128:CH],
                                rs_diag[1][:, 128:CH])
                        elif r8 == 2:
                            ds = rs_diag[2]
                            nc.vector.tensor_add(
                                ds[:, 256:CH], ds[:, 256:CH], pt[:, 256:CH])

                # scores run SKEW blocks ahead of PV so the PE never waits
                # on the exp chain of the block it just scored; oproj
                # matmuls from the previous chunk fill remaining PE slack.
                # Fillers are rationed: head 0 may spend at most half, and
                # each head holds back `tail_res` of its ration to cover the
                # exp-latency-dominated diagonal tail.
                SKEW = CFG["skew"]
                ration = len(fillers) if h == 1 else (len(fillers) + 1) // 2
                tail_res = min(2, ration)
                spend = ration - tail_res
                queue = []
                for jb in range(nj):
                    queue.append((jb, emit_scores(jb)))
                    if fillers and spend > 0:
                        fillers.pop(0)()
                        spend -= 1
                    if len(queue) > SKEW:
                        pj, args = queue.pop(0)
                        emit_pv(pj, *args)
                for pj, args in queue:
                    emit_pv(pj, *args)
                    if fillers and tail_res > 0:
                        fillers.pop(0)()
                        tail_res -= 1
                if fold_diag:
                    ds = rs_diag[2]
                    nc.vector.tensor_add(
                        ds[:, 384:CH], ds[:, 384:CH], rs_diag[4][:, 384:CH])
                    # start=True when no group matmul covered [128:CH) (qi=0)
                    rs_matmul(ds[:, 128:CH], slice(128, CH), True,
                              start=(qi == 0))

                rsb = r.rspool.tile([128, CH], f32, tag="rsb", name="rsb")
                nc.vector.reciprocal(rsb[:], psr[:])
                at = r.atpool.tile([128, CH], bf16, tag="at", name="at")
                nc.vector.tensor_mul(at[:], pso[:], rsb[:])
                at_pair.append(at)
            for f in fillers:
                f()
            fillers = []
            prev_at = at_pair
            prev_qi = qi
        for f in oproj_fillers(b, prev_qi, prev_at):
            f()


class _Res:
    pass


def build_nc(seq_len=S, loop_n=1, parts="all"):
    """Build the per-core program. loop_n>1 wraps the body in a hardware
    loop — a timing-only variant used to measure per-iteration device
    time through the noisy dispatch path."""
    import concourse.mybir as mybir
    import concourse.tile as tile
    from concourse import bacc
    from concourse.masks import make_identity

    bf16 = mybir.dt.bfloat16
    f32 = mybir.dt.float32
    NT = seq_len // 128

    nc = bacc.Bacc("TRN2", target_bir_lowering=False, debug=False)

    r = _Res()
    # x pretiled on the host: [B, chunk, partition, hid-block, token] so a
    # chunk load is 128 contiguous 16KB rows (128 DMA descriptors, not 2048).
    r.xT = nc.dram_tensor("xt", [B, seq_len // CH, 128, HID // 128, CH],
                          bf16, kind="ExternalInput")
    wqk = nc.dram_tensor("wqk", [HID, 4 * DH], bf16, kind="ExternalInput")
    wv = nc.dram_tensor("wv", [HID, 2 * DH], bf16, kind="ExternalInput")
    wo = nc.dram_tensor("wo", [2 * DH, HID], bf16, kind="ExternalInput")
    rope_d = {}
    for nm in ("cosT", "ssinT"):
        rope_d[nm] = nc.dram_tensor(nm, [DH, seq_len], bf16,
                                    kind="ExternalInput")
    mask_d = nc.dram_tensor("masks", [4, 128, CH], bf16, kind="ExternalInput")
    r.out_d = nc.dram_tensor("out", [B, seq_len, HID], bf16,
                             kind="ExternalOutput")

    with tile.TileContext(nc) as tc:
        with (
            tc.tile_pool(name="consts", bufs=1) as cpool,
            tc.tile_pool(name="x", bufs=3) as xpool,
            tc.tile_pool(name="qk", bufs=32) as qkpool,
            tc.tile_pool(name="v", bufs=8) as vpool,
            tc.tile_pool(name="pt", bufs=CFG["pt_bufs"]) as ptpool,
            tc.tile_pool(name="at", bufs=4) as atpool,
            tc.tile_pool(name="tmp", bufs=2) as tpool,
            tc.tile_pool(name="s", bufs=6) as spool,
            tc.tile_pool(name="rs", bufs=2) as rspool,
            tc.tile_pool(name="o", bufs=2) as opool,
            tc.tile_pool(name="psA", bufs=CFG["psa"], space="PSUM") as psA,
            tc.tile_pool(name="psB", bufs=2, space="PSUM") as psB,
            tc.tile_pool(name="psC", bufs=CFG["psc"], space="PSUM") as psC,
            tc.tile_pool(name="psD", bufs=1, space="PSUM") as psD,
        ):
            r.xpool, r.qkpool, r.vpool, r.ptpool = xpool, qkpool, vpool, ptpool
            r.atpool, r.tpool, r.rspool = atpool, tpool, rspool
            r.opool, r.spool = opool, spool
            r.psA, r.psB, r.psC, r.psD = psA, psB, psC, psD

            r.wqk_sb = cpool.tile([128, HID // 128, 4 * DH], bf16, name="wqk_sb")
            nc.sync.dma_start(r.wqk_sb[:], wqk.rearrange("(n p) o -> p n o", p=128))
            r.wv_sb = cpool.tile([128, HID // 128, 2 * DH], bf16, name="wv_sb")
            nc.sync.dma_start(r.wv_sb[:], wv.rearrange("(n p) o -> p n o", p=128))
            r.wo_sb = cpool.tile([128, 2, HID], bf16, name="wo_sb")
            nc.sync.dma_start(r.wo_sb[:], wo.rearrange("(n p) o -> p n o", p=128))
            for nm in ("cosT", "ssinT"):
                t = cpool.tile([128, seq_len], bf16, name=nm)
                nc.sync.dma_start(t[:], rope_d[nm][:, :])
                setattr(r, nm, t)
            r.mask_sb = cpool.tile([128, 4, CH], bf16, name="mask_sb")
            nc.sync.dma_start(r.mask_sb[:], mask_d.rearrange("n p o -> p n o"))
            r.ident = cpool.tile([128, 128], bf16, name="ident")
            make_identity(nc, r.ident[:])
            r.ones = cpool.tile([128, 128], bf16, name="ones")
            nc.gpsimd.memset(r.ones[:], 1.0)

            loop_ctx = (tc.For_i(0, loop_n, 1) if loop_n > 1
                        else contextlib.nullcontext())
            with loop_ctx:
                _emit_body(nc, r, seq_len, parts)

    nc.compile()
    return nc


# ----------------------------------------------------------------------------
# host-side sharding / tables
# ----------------------------------------------------------------------------

def host_tables(seq_len=S):
    bf = ml_dtypes.bfloat16
    inv = 1.0 / (10000.0 ** (np.arange(0, DH, 2, dtype=np.float64) / DH))
    ang = inv[:, None] * np.arange(seq_len, dtype=np.float64)[None, :]  # [64, S]
    cos = np.cos(ang)
    sin = np.sin(ang)
    cosT = np.concatenate([cos, cos], axis=0)                    # [128, S]
    ssinT = np.concatenate([-sin, sin], axis=0)                  # signed swap mult
    tabs = {
        "cosT": np.ascontiguousarray(cosT).astype(bf),
        "ssinT": np.ascontiguousarray(ssinT).astype(bf),
    }
    p = np.arange(128)[:, None]
    f = np.arange(CH)[None, :]
    masks = np.stack(
        [np.where(p + 128 * ri <= f, 0.0, NEG) for ri in range(4)]
    ).astype(bf)
    tabs["masks"] = masks
    return tabs


def host_in_maps(x, w_qkv, w_o, seq_len=S):
    bf = ml_dtypes.bfloat16
    x = np.asarray(x, dtype=np.float32)
    w_qkv = np.asarray(w_qkv, dtype=np.float32)
    w_o = np.asarray(w_o, dtype=np.float32)
    # Pretile x^T as [B, chunk, partition, hid-block, token-in-chunk] so
    # each chunk DMA reads 128 contiguous rows.
    TC = seq_len // CH
    xT = x.transpose(0, 2, 1).reshape(B, HID // 128, 128, TC, CH)
    xT = np.ascontiguousarray(xT.transpose(0, 3, 2, 1, 4)).astype(bf)
    tabs = host_tables(seq_len)
    maps = []
    for c in range(NCORES):
        h0 = HPC * c
        rows = []
        for base in (0, HID):  # q rows, then k rows
            for h in range(h0, h0 + HPC):
                rows.append(w_qkv[base + h * DH:base + (h + 1) * DH])
        wqk_c = np.ascontiguousarray(np.concatenate(rows, axis=0).T).astype(bf)
        vrows = [w_qkv[2 * HID + h * DH:2 * HID + (h + 1) * DH]
                 for h in range(h0, h0 + HPC)]
        wv_c = np.ascontiguousarray(np.concatenate(vrows, axis=0).T).astype(bf)
        wo_c = np.ascontiguousarray(
            w_o[:, h0 * DH:(h0 + HPC) * DH].T).astype(bf)
        maps.append({
            "xt": xT, "wqk": wqk_c, "wv": wv_c, "wo": wo_c,
            "cosT": tabs["cosT"], "ssinT": tabs["ssinT"],
            "masks": tabs["masks"],
        })
    return maps


def kernel(x, w_qkv, w_o):
    from concourse import bass_utils

    if "nc" not in _STATE:
        _STATE["nc"] = build_nc(S)
    nc = _STATE["nc"]
    in_maps = host_in_maps(x, w_qkv, w_o, S)
    res = bass_utils.run_bass_kernel_spmd(
        nc, in_maps, core_ids=list(range(NCORES)))
    out = np.zeros((B, S, HID), dtype=np.float32)
    for r in res.results:
        out += np.asarray(r["out"], dtype=np.float32)
    return out
